# revision 7
# baseline (speedup 1.0000x reference)
"""Causal multi-head attention (B=2, S=2048, D=1024, H=16) on 8 trn2 cores.

Sharding: batch (2-way) x head-group (4-way) = 8 cores. Each core computes
QKV projection for its batch restricted to its 4 heads, causal attention,
and a row-parallel slice of the output projection; the host sums the 4
partial outputs per batch (the all-reduce of the row-parallel Wo matmul).

Per-core kernel (Tile framework, fp32 data / fp32r matmuls):
  - x [2048,1024] is PE-transposed on chip to x^T tiles (contract dim D on
    partitions).
  - Q,K are produced in [feat, seq] layout (rhs = x^T), V in [seq, feat]
    layout (lhsT = x^T) with an extra ones-column per head so the PV matmul
    also produces the softmax denominator.
  - Scores are computed transposed, S_T[key, q] = K_blk.T @ Q (N=512 so
    fp32r runs at full rate), exp on ScalarE (scale=1/8 folded in), causal
    masking via precomputed 0/1 staircase masks on the diagonal blocks.
  - PV: out_T[65, q] = V_aug.T @ exp(S_T), accumulated over key blocks; row
    64 is the denominator. Normalization multiplies by a reciprocal row
    broadcast across partitions with a K=1 outer-product matmul.
  - Wo: out[q, :] = sum_c vw_T_c.T @ Wo_c.
"""

import numpy as np
from contextlib import ExitStack

import concourse.bass as bass
import concourse.mybir as mybir
import concourse.tile as tile
from concourse import bacc
from concourse.bass_utils import run_bass_kernel_spmd
from concourse.masks import make_identity

B, S, D, H, HD = 2, 2048, 1024, 16, 64
NCORES = 8
NHG = 4                  # head groups (cores per batch)
NH = H // NHG            # 4 local heads
FQK = NH * HD * 2        # 512 local q+k features
FV = NH * HD             # 256 local v features
QB = 512                 # query block (attention outer tile)
KB = 128                 # key block
NSC = S // QB            # 4 seq chunks
R32 = mybir.dt.float32r
F32 = mybir.dt.float32
EXP = mybir.ActivationFunctionType.Exp


def _build_body(ctx, tc, x_d, wqk_d, wv_d, bqk_d, bv_d, wo_d, out_d):
    nc = tc.nc

    const = ctx.enter_context(tc.tile_pool(name="const", bufs=1))
    wq_pool = ctx.enter_context(tc.tile_pool(name="wqp", bufs=8))
    wvp = ctx.enter_context(tc.tile_pool(name="wvp", bufs=8))
    wop = ctx.enter_context(tc.tile_pool(name="wop", bufs=2))
    xs_pool = ctx.enter_context(tc.tile_pool(name="xsp", bufs=6))
    xt_pool = ctx.enter_context(tc.tile_pool(name="xtp", bufs=10))
    qk_pool = ctx.enter_context(tc.tile_pool(name="qkp", bufs=1))
    v_pool = ctx.enter_context(tc.tile_pool(name="vp", bufs=16))
    exp_pool = ctx.enter_context(tc.tile_pool(name="ep", bufs=4))
    vw_pool = ctx.enter_context(tc.tile_pool(name="vwp", bufs=2))
    bc_pool = ctx.enter_context(tc.tile_pool(name="bcp", bufs=2))
    rc_pool = ctx.enter_context(tc.tile_pool(name="rcp", bufs=2))
    os_pool = ctx.enter_context(tc.tile_pool(name="osp", bufs=3))
    p1 = ctx.enter_context(tc.tile_pool(name="p1", bufs=2, space="PSUM"))
    ps = ctx.enter_context(tc.tile_pool(name="ps", bufs=2, space="PSUM"))
    po = ctx.enter_context(tc.tile_pool(name="po", bufs=2, space="PSUM"))

    # ---- constants ----
    ident = const.tile([128, 128], F32)
    make_identity(nc, ident)
    ones_row = const.tile([1, 128], R32)
    # Staircase causal masks for the 4 diagonal key-blocks of a 512-wide
    # query chunk: mask_j[k, q] = 1 iff q >= k + 128*j.
    masks = []
    for j in range(4):
        mj = const.tile([128, QB], F32, name=f"mask{j}", tag=f"mask{j}")
        nc.gpsimd.memset(mj, 1.0)
        nc.gpsimd.affine_select(
            out=mj, in_=mj,
            compare_op=mybir.AluOpType.is_ge,
            fill=0.0,
            base=-128 * j,
            pattern=[[1, QB]],
            channel_multiplier=-1,
        )
        masks.append(mj)
    # ones_row = ident*0 + 1 (memset can't write fp32r; tensor_scalar can)
    nc.vector.tensor_scalar(ones_row, ident[0:1, :], 0.0, 1.0,
                            op0=mybir.AluOpType.mult, op1=mybir.AluOpType.add)

    # ---- weights ----
    wqk_sb = []
    for dc in range(8):
        t = wq_pool.tile([128, FQK], R32, name=f"wqk{dc}", tag="wqk")
        nc.sync.dma_start(t, wqk_d.ap()[dc * 128:(dc + 1) * 128, :])
        wqk_sb.append(t)
    wv_sb = []
    for dc in range(8):
        t = wvp.tile([128, FV], R32, name=f"wv{dc}", tag="wv")
        nc.sync.dma_start(t, wv_d.ap()[dc * 128:(dc + 1) * 128, :])
        wv_sb.append(t)
    wo_sb = []
    for c in range(2):
        t = wop.tile([128, D], R32, name=f"wo{c}", tag="wo")
        nc.sync.dma_start(t, wo_d.ap()[c * 128:(c + 1) * 128, :])
        wo_sb.append(t)
    bqk_sb = const.tile([128, 4], F32)
    nc.sync.dma_start(bqk_sb, bqk_d.ap().rearrange("(f p) -> p f", p=128))
    bv_sb = const.tile([1, FV], R32)
    nc.sync.dma_start(bv_sb, bv_d.ap().rearrange("(o e) -> o e", o=1))
    # v-bias broadcast across partitions: [128, FV] = ones[1,128].T @ bv[1,FV]
    bvb_ps = p1.tile([128, FV], F32, name="bvb_ps", tag="p1")
    nc.tensor.matmul(bvb_ps, ones_row, bv_sb, start=True, stop=True)
    bvb_sb = const.tile([128, FV], F32)
    nc.vector.tensor_copy(bvb_sb, bvb_ps)

    # ---- phase B: x transpose + QKV projection ----
    qkT = [qk_pool.tile([128, S], R32, name=f"qkT{f}", tag=f"qkT{f}", bufs=1)
           for f in range(4)]
    v_tiles = []
    for sc in range(NSC):
        xts = []
        for sb in range(4):
            xsb = xs_pool.tile([128, D], F32, name="xsb", tag="xsb")
            nc.sync.dma_start(
                xsb, x_d.ap()[sc * QB + sb * 128: sc * QB + (sb + 1) * 128, :])
            xts.append(xsb)
        xT = []
        for dc in range(8):
            xt = xt_pool.tile([128, QB], R32, name="xt", tag="xt")
            for sb in range(4):
                pt = p1.tile([128, 128], F32, name="pt", tag="p1")
                nc.tensor.transpose(pt, xts[sb][:, dc * 128:(dc + 1) * 128], ident)
                nc.vector.tensor_copy(xt[:, sb * 128:(sb + 1) * 128], pt)
            xT.append(xt)
        # Q,K in [feat, seq]: psum += Wqk_chunk.T @ x^T
        for f in range(4):
            pq = p1.tile([128, QB], F32, name="pq", tag="p1")
            for dc in range(8):
                nc.tensor.matmul(pq, wqk_sb[dc][:, f * 128:(f + 1) * 128],
                                 xT[dc], start=(dc == 0), stop=(dc == 7))
            nc.vector.tensor_scalar_add(
                qkT[f][:, sc * QB:(sc + 1) * QB], pq, bqk_sb[:, f:f + 1])
        # V in [seq, feat]: psum += (x^T_blk).T @ Wv_chunk, plus ones column
        for sb in range(4):
            pv = p1.tile([128, FV], F32, name="pv", tag="p1")
            for dc in range(8):
                nc.tensor.matmul(pv, xT[dc][:, sb * 128:(sb + 1) * 128],
                                 wv_sb[dc], start=(dc == 0), stop=(dc == 7))
            vt = v_pool.tile([128, NH, HD + 1], R32, name="vt", tag="vt")
            nc.vector.tensor_add(vt[:, :, 0:HD],
                                 pv.rearrange("p (h e) -> p h e", h=NH),
                                 bvb_sb.rearrange("p (h e) -> p h e", h=NH))
            nc.vector.tensor_scalar(vt[:, :, HD:HD + 1], vt[:, :, 0:1], 0.0,
                                    1.0, op0=mybir.AluOpType.mult,
                                    op1=mybir.AluOpType.add)
            v_tiles.append(vt)

    # ---- phase C: attention + output projection ----
    for qi in range(NSC):
        vwT = [vw_pool.tile([128, QB], R32, name=f"vwT{c}", tag=f"vwT{c}")
               for c in range(2)]
        for hp in range(2):
            pair = (2 * hp, 2 * hp + 1)
            nkb = (qi + 1) * 4
            poh, Q, Kt = {}, {}, {}
            for h in pair:
                poh[h] = po.tile([HD + 1, QB], F32, name="poh", tag="po")
                r0 = (h % 2) * 64
                Q[h] = qkT[h // 2][r0:r0 + 64, qi * QB:(qi + 1) * QB]
                Kt[h] = qkT[2 + h // 2][r0:r0 + 64, :]
            for base in range(0, nkb, 2):
                es = {}
                for h in pair:
                    psn = ps.tile([128, 2 * QB], F32, name="psn", tag="ps")
                    for j2 in range(2):
                        kb = base + j2
                        nc.tensor.matmul(psn[:, j2 * QB:(j2 + 1) * QB],
                                         Kt[h][:, kb * KB:(kb + 1) * KB],
                                         Q[h], start=True, stop=True)
                    e = exp_pool.tile([128, 2 * QB], R32, name="et", tag="et")
                    nc.scalar.activation(e, psn, EXP, scale=1.0 / np.sqrt(HD))
                    for j2 in range(2):
                        kb = base + j2
                        if kb >= qi * 4:
                            j = kb - qi * 4
                            nc.vector.tensor_mul(e[:, j2 * QB:(j2 + 1) * QB],
                                                 e[:, j2 * QB:(j2 + 1) * QB],
                                                 masks[j])
                    es[h] = e
                for j2 in range(2):
                    kb = base + j2
                    for h in pair:
                        nc.tensor.matmul(poh[h], v_tiles[kb][:, h, :],
                                         es[h][:, j2 * QB:(j2 + 1) * QB],
                                         start=(kb == 0), stop=(kb == nkb - 1))
            for h in pair:
                rc = rc_pool.tile([1, QB], F32, name="rc", tag="rc")
                nc.vector.reciprocal(rc, poh[h][HD:HD + 1, :])
                rc32 = rc_pool.tile([1, QB], R32, name="rc32", tag="rc32")
                nc.vector.tensor_copy(rc32, rc)
                pb = p1.tile([64, QB], F32, name="pb", tag="p1")
                nc.tensor.matmul(pb, ones_row[:, 0:64], rc32,
                                 start=True, stop=True)
                bcs = bc_pool.tile([64, QB], F32, name="bcs", tag="bcs")
                nc.vector.tensor_copy(bcs, pb)
                r0 = (h % 2) * 64
                nc.vector.tensor_mul(vwT[h // 2][r0:r0 + 64, :],
                                     poh[h][0:HD, :], bcs)
        for ql in range(4):
            for do in range(2):
                pw = p1.tile([128, QB], F32, name="pw", tag="p1")
                for c in range(2):
                    nc.tensor.matmul(pw, vwT[c][:, ql * 128:(ql + 1) * 128],
                                     wo_sb[c][:, do * QB:(do + 1) * QB],
                                     start=(c == 0), stop=(c == 1))
                osb = os_pool.tile([128, QB], F32, name="osb", tag="osb")
                nc.vector.tensor_copy(osb, pw)
                nc.sync.dma_start(
                    out_d.ap()[qi * QB + ql * 128: qi * QB + (ql + 1) * 128,
                               do * QB:(do + 1) * QB], osb)


_COMPILED = None


def get_compiled():
    global _COMPILED
    if _COMPILED is not None:
        return _COMPILED
    nc = bacc.Bacc("TRN2", target_bir_lowering=False, debug=False,
                   enable_asserts=False, num_devices=NCORES)
    x_d = nc.dram_tensor("x", [S, D], F32, kind="ExternalInput")
    wqk_d = nc.dram_tensor("wqk", [D, FQK], R32, kind="ExternalInput")
    wv_d = nc.dram_tensor("wv", [D, FV], R32, kind="ExternalInput")
    bqk_d = nc.dram_tensor("bqk", [FQK], F32, kind="ExternalInput")
    bv_d = nc.dram_tensor("bv", [FV], R32, kind="ExternalInput")
    wo_d = nc.dram_tensor("wo", [FV, D], R32, kind="ExternalInput")
    out_d = nc.dram_tensor("out", [S, D], F32, kind="ExternalOutput")
    with tile.TileContext(nc) as tc:
        with ExitStack() as ctx:
            _build_body(ctx, tc, x_d, wqk_d, wv_d, bqk_d, bv_d, wo_d, out_d)
    nc.compile()
    _COMPILED = nc
    return nc


def make_in_maps(x, Wqkv, bqkv, Wo):
    x = np.ascontiguousarray(np.asarray(x, dtype=np.float32))
    Wqkv = np.asarray(Wqkv, dtype=np.float32)
    bqkv = np.asarray(bqkv, dtype=np.float32)
    Wo = np.asarray(Wo, dtype=np.float32)
    in_maps = []
    for c in range(NCORES):
        b, hg = divmod(c, NHG)
        qs = slice(hg * FV, (hg + 1) * FV)
        ks = slice(D + hg * FV, D + (hg + 1) * FV)
        vs = slice(2 * D + hg * FV, 2 * D + (hg + 1) * FV)
        in_maps.append({
            "x": np.ascontiguousarray(x[b]),
            "wqk": np.ascontiguousarray(
                np.concatenate([Wqkv[:, qs], Wqkv[:, ks]], axis=1)),
            "wv": np.ascontiguousarray(Wqkv[:, vs]),
            "bqk": np.ascontiguousarray(
                np.concatenate([bqkv[qs], bqkv[ks]])),
            "bv": np.ascontiguousarray(bqkv[vs]),
            "wo": np.ascontiguousarray(Wo[hg * FV:(hg + 1) * FV, :]),
        })
    return in_maps


def run_sharded(x, Wqkv, bqkv, Wo, bo, **spmd_kwargs):
    nc = get_compiled()
    in_maps = make_in_maps(x, Wqkv, bqkv, Wo)
    res = run_bass_kernel_spmd(nc, in_maps, core_ids=list(range(NCORES)),
                               **spmd_kwargs)
    out = np.zeros((B, S, D), np.float32)
    for c in range(NCORES):
        out[c // NHG] += res.results[c]["out"]
    out += np.asarray(bo, dtype=np.float32)
    return out, res


def kernel(x, mask, Wqkv, bqkv, Wo, bo):
    out, _ = run_sharded(x, Wqkv, bqkv, Wo, bo)
    return out


# revision 11
# speedup vs baseline: 1.0470x; 1.0470x over previous
"""Causal multi-head attention (B=2, S=2048, D=1024, H=16) on 8 trn2 cores.

Sharding: batch (2-way) x head-group (4-way) = 8 cores. Each core computes
QKV projection for its batch restricted to its 4 heads, causal attention,
and a row-parallel slice of the output projection; the host sums the 4
partial outputs per batch (the all-reduce of the row-parallel Wo matmul).

Per-core kernel (Tile framework, fp32 data / fp32r matmuls):
  - x [2048,1024] is PE-transposed on chip to x^T tiles (contract dim D on
    partitions).
  - Q,K are produced in [feat, seq] layout (rhs = x^T), V in [seq, feat]
    layout (lhsT = x^T) with an extra ones-column per head so the PV matmul
    also produces the softmax denominator.
  - Scores are computed transposed, S_T[key, q] = K_blk.T @ Q (N=512 so
    fp32r runs at full rate), exp on ScalarE (scale=1/8 folded in), causal
    masking via precomputed 0/1 staircase masks on the diagonal blocks.
  - PV: out_T[65, q] = V_aug.T @ exp(S_T), accumulated over key blocks; row
    64 is the denominator. Normalization multiplies by a reciprocal row
    broadcast across partitions with a K=1 outer-product matmul.
  - Wo: out[q, :] = sum_c vw_T_c.T @ Wo_c.
"""

import numpy as np
from contextlib import ExitStack

import concourse.bass as bass
import concourse.mybir as mybir
import concourse.tile as tile
from concourse import bacc
from concourse.bass_utils import run_bass_kernel_spmd
from concourse.masks import make_identity

B, S, D, H, HD = 2, 2048, 1024, 16, 64
NCORES = 8
NHG = 4                  # head groups (cores per batch)
NH = H // NHG            # 4 local heads
FQK = NH * HD * 2        # 512 local q+k features
FV = NH * HD             # 256 local v features
QB = 512                 # query block (attention outer tile)
KB = 128                 # key block
NSC = S // QB            # 4 seq chunks
R32 = mybir.dt.float32r
F32 = mybir.dt.float32
EXP = mybir.ActivationFunctionType.Exp


def _build_body(ctx, tc, x_d, wqk_d, wv_d, bqk_d, bv_d, wo_d, out_d):
    nc = tc.nc

    const = ctx.enter_context(tc.tile_pool(name="const", bufs=1))
    wq_pool = ctx.enter_context(tc.tile_pool(name="wqp", bufs=8))
    wvp = ctx.enter_context(tc.tile_pool(name="wvp", bufs=8))
    wop = ctx.enter_context(tc.tile_pool(name="wop", bufs=2))
    xs_pool = ctx.enter_context(tc.tile_pool(name="xsp", bufs=6))
    xt_pool = ctx.enter_context(tc.tile_pool(name="xtp", bufs=10))
    qk_pool = ctx.enter_context(tc.tile_pool(name="qkp", bufs=1))
    v_pool = ctx.enter_context(tc.tile_pool(name="vp", bufs=16))
    exp_pool = ctx.enter_context(tc.tile_pool(name="ep", bufs=4))
    vw_pool = ctx.enter_context(tc.tile_pool(name="vwp", bufs=2))
    rc_pool = ctx.enter_context(tc.tile_pool(name="rcp", bufs=2))
    os_pool = ctx.enter_context(tc.tile_pool(name="osp", bufs=3))
    p1 = ctx.enter_context(tc.tile_pool(name="p1", bufs=2, space="PSUM"))
    ps = ctx.enter_context(tc.tile_pool(name="ps", bufs=2, space="PSUM"))
    po = ctx.enter_context(tc.tile_pool(name="po", bufs=2, space="PSUM"))

    # ---- constants ----
    ident = const.tile([128, 128], F32)
    make_identity(nc, ident)
    ones_row = const.tile([1, 128], R32)
    # Staircase causal masks for the 4 diagonal key-blocks of a 512-wide
    # query chunk: mask_j[k, q] = 1 iff q >= k + 128*j.
    masks = []
    for j in range(4):
        mj = const.tile([128, QB], F32, name=f"mask{j}", tag=f"mask{j}")
        nc.gpsimd.memset(mj, 1.0)
        nc.gpsimd.affine_select(
            out=mj, in_=mj,
            compare_op=mybir.AluOpType.is_ge,
            fill=0.0,
            base=-128 * j,
            pattern=[[1, QB]],
            channel_multiplier=-1,
        )
        masks.append(mj)
    # ones_row = ident*0 + 1 (memset can't write fp32r; tensor_scalar can)
    nc.vector.tensor_scalar(ones_row, ident[0:1, :], 0.0, 1.0,
                            op0=mybir.AluOpType.mult, op1=mybir.AluOpType.add)

    # ---- weights ----
    wqk_sb = []
    for dc in range(8):
        t = wq_pool.tile([128, FQK], R32, name=f"wqk{dc}", tag="wqk")
        nc.sync.dma_start(t, wqk_d.ap()[dc * 128:(dc + 1) * 128, :])
        wqk_sb.append(t)
    wv_sb = []
    for dc in range(8):
        t = wvp.tile([128, FV], R32, name=f"wv{dc}", tag="wv")
        nc.sync.dma_start(t, wv_d.ap()[dc * 128:(dc + 1) * 128, :])
        wv_sb.append(t)
    wo_sb = []
    for c in range(2):
        t = wop.tile([128, D], R32, name=f"wo{c}", tag="wo")
        nc.sync.dma_start(t, wo_d.ap()[c * 128:(c + 1) * 128, :])
        wo_sb.append(t)
    bqk_sb = const.tile([128, 4], F32)
    nc.sync.dma_start(bqk_sb, bqk_d.ap().rearrange("(f p) -> p f", p=128))
    bv_sb = const.tile([1, FV], R32)
    nc.sync.dma_start(bv_sb, bv_d.ap().rearrange("(o e) -> o e", o=1))
    # v-bias broadcast across partitions: [128, FV] = ones[1,128].T @ bv[1,FV]
    bvb_ps = p1.tile([128, FV], F32, name="bvb_ps", tag="p1")
    nc.tensor.matmul(bvb_ps, ones_row, bv_sb, start=True, stop=True)
    bvb_sb = const.tile([128, FV], F32)
    nc.vector.tensor_copy(bvb_sb, bvb_ps)

    # ---- phase B: x transpose + QKV projection ----
    qkT = [qk_pool.tile([128, S], R32, name=f"qkT{f}", tag=f"qkT{f}", bufs=1)
           for f in range(4)]
    v_tiles = []
    for sc in range(NSC):
        xts = []
        for sb in range(4):
            xsb = xs_pool.tile([128, D], F32, name="xsb", tag="xsb")
            nc.sync.dma_start(
                xsb, x_d.ap()[sc * QB + sb * 128: sc * QB + (sb + 1) * 128, :])
            xts.append(xsb)
        xT = []
        for dc in range(8):
            xt = xt_pool.tile([128, QB], R32, name="xt", tag="xt")
            for sb in range(4):
                pt = p1.tile([128, 128], F32, name="pt", tag="p1")
                nc.tensor.transpose(pt, xts[sb][:, dc * 128:(dc + 1) * 128], ident)
                nc.vector.tensor_copy(xt[:, sb * 128:(sb + 1) * 128], pt)
            xT.append(xt)
        # Q,K in [feat, seq]: psum += Wqk_chunk.T @ x^T
        for f in range(4):
            pq = p1.tile([128, QB], F32, name="pq", tag="p1")
            for dc in range(8):
                nc.tensor.matmul(pq, wqk_sb[dc][:, f * 128:(f + 1) * 128],
                                 xT[dc], start=(dc == 0), stop=(dc == 7))
            nc.vector.tensor_scalar_add(
                qkT[f][:, sc * QB:(sc + 1) * QB], pq, bqk_sb[:, f:f + 1])
        # V in [seq, feat]: psum += (x^T_blk).T @ Wv_chunk, plus ones column
        for sb in range(4):
            pv = p1.tile([128, FV], F32, name="pv", tag="p1")
            for dc in range(8):
                nc.tensor.matmul(pv, xT[dc][:, sb * 128:(sb + 1) * 128],
                                 wv_sb[dc], start=(dc == 0), stop=(dc == 7))
            vt = v_pool.tile([128, NH, HD + 1], R32, name="vt", tag="vt")
            nc.vector.tensor_add(vt[:, :, 0:HD],
                                 pv.rearrange("p (h e) -> p h e", h=NH),
                                 bvb_sb.rearrange("p (h e) -> p h e", h=NH))
            nc.vector.tensor_scalar(vt[:, :, HD:HD + 1], vt[:, :, 0:1], 0.0,
                                    1.0, op0=mybir.AluOpType.mult,
                                    op1=mybir.AluOpType.add)
            v_tiles.append(vt)

    # ---- phase C: attention + output projection ----
    for qi in range(NSC):
        vwT = [vw_pool.tile([128, QB], R32, name=f"vwT{c}", tag=f"vwT{c}")
               for c in range(2)]
        for hp in range(2):
            pair = (2 * hp, 2 * hp + 1)
            nkb = (qi + 1) * 4
            poh, Q, Kt = {}, {}, {}
            for h in pair:
                poh[h] = po.tile([HD + 1, QB], F32, name="poh", tag="po")
                r0 = (h % 2) * 64
                Q[h] = qkT[h // 2][r0:r0 + 64, qi * QB:(qi + 1) * QB]
                Kt[h] = qkT[2 + h // 2][r0:r0 + 64, :]
            for base in range(0, nkb, 2):
                es = {}
                for h in pair:
                    psn = ps.tile([128, 2 * QB], F32, name="psn", tag="ps")
                    for j2 in range(2):
                        kb = base + j2
                        nc.tensor.matmul(psn[:, j2 * QB:(j2 + 1) * QB],
                                         Kt[h][:, kb * KB:(kb + 1) * KB],
                                         Q[h], start=True, stop=True)
                    e = exp_pool.tile([128, 2 * QB], R32, name="et", tag="et")
                    nc.scalar.activation(e, psn, EXP, scale=1.0 / np.sqrt(HD))
                    for j2 in range(2):
                        kb = base + j2
                        if kb >= qi * 4:
                            j = kb - qi * 4
                            nc.vector.tensor_mul(e[:, j2 * QB:(j2 + 1) * QB],
                                                 e[:, j2 * QB:(j2 + 1) * QB],
                                                 masks[j])
                    es[h] = e
                for j2 in range(2):
                    kb = base + j2
                    for h in pair:
                        nc.tensor.matmul(poh[h], v_tiles[kb][:, h, :],
                                         es[h][:, j2 * QB:(j2 + 1) * QB],
                                         start=(kb == 0), stop=(kb == nkb - 1))
            for h in pair:
                sum_sb = rc_pool.tile([1, QB], F32, name="sum_sb", tag="sum_sb")
                nc.vector.tensor_copy(sum_sb, poh[h][HD:HD + 1, :])
                rc = rc_pool.tile([1, QB], F32, name="rc", tag="rc")
                nc.vector.reciprocal_approx_fast(rc, sum_sb)
                rc32 = rc_pool.tile([1, QB], R32, name="rc32", tag="rc32")
                nc.vector.tensor_copy(rc32, rc)
                pb = p1.tile([64, QB], F32, name="pb", tag="p1")
                nc.tensor.matmul(pb, ones_row[:, 0:64], rc32,
                                 start=True, stop=True)
                bcs = rc_pool.tile([64, QB], F32, name="bcs", tag="bcs")
                nc.vector.tensor_copy(bcs, pb)
                r0 = (h % 2) * 64
                nc.vector.tensor_mul(vwT[h // 2][r0:r0 + 64, :],
                                     poh[h][0:HD, :], bcs)
        for ql in range(4):
            for do in range(2):
                pw = p1.tile([128, QB], F32, name="pw", tag="p1")
                for c in range(2):
                    nc.tensor.matmul(pw, vwT[c][:, ql * 128:(ql + 1) * 128],
                                     wo_sb[c][:, do * QB:(do + 1) * QB],
                                     start=(c == 0), stop=(c == 1))
                osb = os_pool.tile([128, QB], F32, name="osb", tag="osb")
                nc.vector.tensor_copy(osb, pw)
                nc.sync.dma_start(
                    out_d.ap()[qi * QB + ql * 128: qi * QB + (ql + 1) * 128,
                               do * QB:(do + 1) * QB], osb)


_COMPILED = None


def get_compiled():
    global _COMPILED
    if _COMPILED is not None:
        return _COMPILED
    nc = bacc.Bacc("TRN2", target_bir_lowering=False, debug=False,
                   enable_asserts=False, num_devices=NCORES)
    x_d = nc.dram_tensor("x", [S, D], F32, kind="ExternalInput")
    wqk_d = nc.dram_tensor("wqk", [D, FQK], R32, kind="ExternalInput")
    wv_d = nc.dram_tensor("wv", [D, FV], R32, kind="ExternalInput")
    bqk_d = nc.dram_tensor("bqk", [FQK], F32, kind="ExternalInput")
    bv_d = nc.dram_tensor("bv", [FV], R32, kind="ExternalInput")
    wo_d = nc.dram_tensor("wo", [FV, D], R32, kind="ExternalInput")
    out_d = nc.dram_tensor("out", [S, D], F32, kind="ExternalOutput")
    with tile.TileContext(nc) as tc:
        with ExitStack() as ctx:
            _build_body(ctx, tc, x_d, wqk_d, wv_d, bqk_d, bv_d, wo_d, out_d)
    nc.compile()
    _COMPILED = nc
    return nc


def make_in_maps(x, Wqkv, bqkv, Wo):
    x = np.ascontiguousarray(np.asarray(x, dtype=np.float32))
    Wqkv = np.asarray(Wqkv, dtype=np.float32)
    bqkv = np.asarray(bqkv, dtype=np.float32)
    Wo = np.asarray(Wo, dtype=np.float32)
    in_maps = []
    for c in range(NCORES):
        b, hg = divmod(c, NHG)
        qs = slice(hg * FV, (hg + 1) * FV)
        ks = slice(D + hg * FV, D + (hg + 1) * FV)
        vs = slice(2 * D + hg * FV, 2 * D + (hg + 1) * FV)
        in_maps.append({
            "x": np.ascontiguousarray(x[b]),
            "wqk": np.ascontiguousarray(
                np.concatenate([Wqkv[:, qs], Wqkv[:, ks]], axis=1)),
            "wv": np.ascontiguousarray(Wqkv[:, vs]),
            "bqk": np.ascontiguousarray(
                np.concatenate([bqkv[qs], bqkv[ks]])),
            "bv": np.ascontiguousarray(bqkv[vs]),
            "wo": np.ascontiguousarray(Wo[hg * FV:(hg + 1) * FV, :]),
        })
    return in_maps


def run_sharded(x, Wqkv, bqkv, Wo, bo, **spmd_kwargs):
    nc = get_compiled()
    in_maps = make_in_maps(x, Wqkv, bqkv, Wo)
    res = run_bass_kernel_spmd(nc, in_maps, core_ids=list(range(NCORES)),
                               **spmd_kwargs)
    out = np.zeros((B, S, D), np.float32)
    for c in range(NCORES):
        out[c // NHG] += res.results[c]["out"]
    out += np.asarray(bo, dtype=np.float32)
    return out, res


def kernel(x, mask, Wqkv, bqkv, Wo, bo):
    out, _ = run_sharded(x, Wqkv, bqkv, Wo, bo)
    return out


# revision 13
# speedup vs baseline: 1.2464x; 1.1904x over previous
"""Causal multi-head attention (B=2, S=2048, D=1024, H=16) on 8 trn2 cores.

Sharding: batch (2-way) x head-group (4-way) = 8 cores. Each core computes
QKV projection for its batch restricted to its 4 heads, causal attention,
and a row-parallel slice of the output projection; the host sums the 4
partial outputs per batch (the all-reduce of the row-parallel Wo matmul).

Per-core kernel (Tile framework, fp32 data / fp32r matmuls):
  - x [2048,1024] is PE-transposed on chip to x^T tiles (contract dim D on
    partitions).
  - Q,K are produced in [feat, seq] layout (rhs = x^T), V in [seq, feat]
    layout (lhsT = x^T) with an extra ones-column per head so the PV matmul
    also produces the softmax denominator.
  - Scores are computed transposed, S_T[key, q] = K_blk.T @ Q (N=512 so
    fp32r runs at full rate), exp on ScalarE (scale=1/8 folded in), causal
    masking via precomputed 0/1 staircase masks on the diagonal blocks.
  - PV: out_T[65, q] = V_aug.T @ exp(S_T), accumulated over key blocks; row
    64 is the denominator. Normalization multiplies by a reciprocal row
    broadcast across partitions with a K=1 outer-product matmul.
  - Wo: out[q, :] = sum_c vw_T_c.T @ Wo_c.
"""

import numpy as np
from contextlib import ExitStack

import concourse.bass as bass
import concourse.mybir as mybir
import concourse.tile as tile
from concourse import bacc
from concourse.bass_utils import run_bass_kernel_spmd
from concourse.masks import make_identity

B, S, D, H, HD = 2, 2048, 1024, 16, 64
NCORES = 8
NHG = 4                  # head groups (cores per batch)
NH = H // NHG            # 4 local heads
FQK = NH * HD * 2        # 512 local q+k features
FV = NH * HD             # 256 local v features
QB = 512                 # query block (attention outer tile)
KB = 128                 # key block
NSC = S // QB            # 4 seq chunks
R32 = mybir.dt.float32r
B16 = mybir.dt.bfloat16
F16 = mybir.dt.float16
F32 = mybir.dt.float32
EXP = mybir.ActivationFunctionType.Exp


def _build_body(ctx, tc, x_d, wqk_d, wv_d, bqk_d, bv_d, wo_d, out_d):
    nc = tc.nc

    const = ctx.enter_context(tc.tile_pool(name="const", bufs=1))
    wq_pool = ctx.enter_context(tc.tile_pool(name="wqp", bufs=8))
    wvp = ctx.enter_context(tc.tile_pool(name="wvp", bufs=8))
    wop = ctx.enter_context(tc.tile_pool(name="wop", bufs=2))
    xs_pool = ctx.enter_context(tc.tile_pool(name="xsp", bufs=6))
    xt_pool = ctx.enter_context(tc.tile_pool(name="xtp", bufs=10))
    qk_pool = ctx.enter_context(tc.tile_pool(name="qkp", bufs=1))
    v_pool = ctx.enter_context(tc.tile_pool(name="vp", bufs=16))
    exp_pool = ctx.enter_context(tc.tile_pool(name="ep", bufs=4))
    vw_pool = ctx.enter_context(tc.tile_pool(name="vwp", bufs=2))
    rc_pool = ctx.enter_context(tc.tile_pool(name="rcp", bufs=2))
    os_pool = ctx.enter_context(tc.tile_pool(name="osp", bufs=3))
    p1 = ctx.enter_context(tc.tile_pool(name="p1", bufs=2, space="PSUM"))
    ps = ctx.enter_context(tc.tile_pool(name="ps", bufs=2, space="PSUM"))
    po = ctx.enter_context(tc.tile_pool(name="po", bufs=2, space="PSUM"))

    # ---- constants ----
    ident = const.tile([128, 128], F32)
    make_identity(nc, ident)
    ones_row = const.tile([1, 128], R32)
    # Staircase causal masks for the 4 diagonal key-blocks of a 512-wide
    # query chunk: mask_j[k, q] = 1 iff q >= k + 128*j.
    masks = []
    for j in range(4):
        mj = const.tile([128, QB], F16, name=f"mask{j}", tag=f"mask{j}")
        nc.gpsimd.memset(mj, 1.0)
        nc.gpsimd.affine_select(
            out=mj, in_=mj,
            compare_op=mybir.AluOpType.is_ge,
            fill=0.0,
            base=-128 * j,
            pattern=[[1, QB]],
            channel_multiplier=-1,
        )
        masks.append(mj)
    # ones_row = ident*0 + 1 (memset can't write fp32r; tensor_scalar can)
    nc.vector.tensor_scalar(ones_row, ident[0:1, :], 0.0, 1.0,
                            op0=mybir.AluOpType.mult, op1=mybir.AluOpType.add)

    # ---- first x chunk before weights: PE transposes depend only on x ----
    xts0 = []
    for sb in range(4):
        xsb = xs_pool.tile([128, D], F32, name="xsb", tag="xsb")
        nc.sync.dma_start(xsb, x_d.ap()[sb * 128:(sb + 1) * 128, :])
        xts0.append(xsb)

    # ---- weights ----
    wqk_sb = []
    for dc in range(8):
        t = wq_pool.tile([128, FQK], R32, name=f"wqk{dc}", tag="wqk")
        nc.sync.dma_start(t, wqk_d.ap()[dc * 128:(dc + 1) * 128, :])
        wqk_sb.append(t)
    wv_sb = []
    for dc in range(8):
        t = wvp.tile([128, FV], R32, name=f"wv{dc}", tag="wv")
        nc.sync.dma_start(t, wv_d.ap()[dc * 128:(dc + 1) * 128, :])
        wv_sb.append(t)
    wo_sb = []
    for c in range(2):
        t = wop.tile([128, D], R32, name=f"wo{c}", tag="wo")
        nc.sync.dma_start(t, wo_d.ap()[c * 128:(c + 1) * 128, :])
        wo_sb.append(t)
    bqk_sb = const.tile([128, 4], F32)
    nc.sync.dma_start(bqk_sb, bqk_d.ap().rearrange("(f p) -> p f", p=128))
    bv_sb = const.tile([1, FV], R32)
    nc.sync.dma_start(bv_sb, bv_d.ap().rearrange("(o e) -> o e", o=1))
    # v-bias broadcast across partitions: [128, FV] = ones[1,128].T @ bv[1,FV]
    bvb_ps = p1.tile([128, FV], F32, name="bvb_ps", tag="p1")
    nc.tensor.matmul(bvb_ps, ones_row, bv_sb, start=True, stop=True)
    bvb_sb = const.tile([128, FV], F32)
    nc.vector.tensor_copy(bvb_sb, bvb_ps)

    # ---- phase B: x transpose + QKV projection ----
    qkT = [qk_pool.tile([128, S], F16, name=f"qkT{f}", tag=f"qkT{f}", bufs=1)
           for f in range(4)]
    v_tiles = []
    for sc in range(NSC):
        if sc == 0:
            xts = xts0
        else:
            xts = []
            for sb in range(4):
                xsb = xs_pool.tile([128, D], F32, name="xsb", tag="xsb")
                nc.sync.dma_start(
                    xsb,
                    x_d.ap()[sc * QB + sb * 128: sc * QB + (sb + 1) * 128, :])
                xts.append(xsb)
        xT = []
        for dc in range(8):
            xt = xt_pool.tile([128, QB], R32, name="xt", tag="xt")
            for sb in range(4):
                pt = p1.tile([128, 128], F32, name="pt", tag="p1")
                nc.tensor.transpose(pt, xts[sb][:, dc * 128:(dc + 1) * 128], ident)
                nc.vector.tensor_copy(xt[:, sb * 128:(sb + 1) * 128], pt)
            xT.append(xt)
        # Q,K in [feat, seq]: psum += Wqk_chunk.T @ x^T
        for f in range(4):
            pq = p1.tile([128, QB], F32, name="pq", tag="p1")
            for dc in range(8):
                nc.tensor.matmul(pq, wqk_sb[dc][:, f * 128:(f + 1) * 128],
                                 xT[dc], start=(dc == 0), stop=(dc == 7))
            nc.vector.tensor_scalar_add(
                qkT[f][:, sc * QB:(sc + 1) * QB], pq, bqk_sb[:, f:f + 1])
        # V in [seq, feat]: psum += (x^T_blk).T @ Wv_chunk, plus ones column
        for sb in range(4):
            pv = p1.tile([128, FV], F32, name="pv", tag="p1")
            for dc in range(8):
                nc.tensor.matmul(pv, xT[dc][:, sb * 128:(sb + 1) * 128],
                                 wv_sb[dc], start=(dc == 0), stop=(dc == 7))
            vt = v_pool.tile([128, NH, HD + 1], F16, name="vt", tag="vt")
            nc.vector.tensor_add(vt[:, :, 0:HD],
                                 pv.rearrange("p (h e) -> p h e", h=NH),
                                 bvb_sb.rearrange("p (h e) -> p h e", h=NH))
            nc.vector.tensor_scalar(vt[:, :, HD:HD + 1], vt[:, :, 0:1], 0.0,
                                    1.0, op0=mybir.AluOpType.mult,
                                    op1=mybir.AluOpType.add)
            v_tiles.append(vt)

    # ---- phase C: attention + output projection ----
    for qi in range(NSC):
        vwT = [vw_pool.tile([128, QB], R32, name=f"vwT{c}", tag=f"vwT{c}")
               for c in range(2)]
        for hp in range(2):
            pair = (2 * hp, 2 * hp + 1)
            nkb = (qi + 1) * 4
            poh, Q, Kt = {}, {}, {}
            for h in pair:
                poh[h] = po.tile([HD + 1, QB], F32, name="poh", tag="po")
                r0 = (h % 2) * 64
                Q[h] = qkT[h // 2][r0:r0 + 64, qi * QB:(qi + 1) * QB]
                Kt[h] = qkT[2 + h // 2][r0:r0 + 64, :]
            for base in range(0, nkb, 2):
                es = {}
                for h in pair:
                    psn = ps.tile([128, 2 * QB], F32, name="psn", tag="ps")
                    for j2 in range(2):
                        kb = base + j2
                        nc.tensor.matmul(psn[:, j2 * QB:(j2 + 1) * QB],
                                         Kt[h][:, kb * KB:(kb + 1) * KB],
                                         Q[h], start=True, stop=True)
                    e = exp_pool.tile([128, 2 * QB], F16, name="et", tag="et")
                    nc.scalar.activation(e, psn, EXP, scale=1.0 / np.sqrt(HD))
                    for j2 in range(2):
                        kb = base + j2
                        if kb >= qi * 4:
                            j = kb - qi * 4
                            nc.vector.tensor_mul(e[:, j2 * QB:(j2 + 1) * QB],
                                                 e[:, j2 * QB:(j2 + 1) * QB],
                                                 masks[j])
                    es[h] = e
                for j2 in range(2):
                    kb = base + j2
                    for h in pair:
                        nc.tensor.matmul(poh[h], v_tiles[kb][:, h, :],
                                         es[h][:, j2 * QB:(j2 + 1) * QB],
                                         start=(kb == 0), stop=(kb == nkb - 1))
            for h in pair:
                sum_sb = rc_pool.tile([1, QB], F32, name="sum_sb", tag="sum_sb")
                nc.vector.tensor_copy(sum_sb, poh[h][HD:HD + 1, :])
                rc = rc_pool.tile([1, QB], F32, name="rc", tag="rc")
                nc.vector.reciprocal_approx_fast(rc, sum_sb)
                rc32 = rc_pool.tile([1, QB], R32, name="rc32", tag="rc32")
                nc.vector.tensor_copy(rc32, rc)
                pb = p1.tile([64, QB], F32, name="pb", tag="p1")
                nc.tensor.matmul(pb, ones_row[:, 0:64], rc32,
                                 start=True, stop=True)
                bcs = rc_pool.tile([64, QB], F32, name="bcs", tag="bcs")
                nc.vector.tensor_copy(bcs, pb)
                r0 = (h % 2) * 64
                nc.vector.tensor_mul(vwT[h // 2][r0:r0 + 64, :],
                                     poh[h][0:HD, :], bcs)
        for ql in range(4):
            for do in range(2):
                pw = p1.tile([128, QB], F32, name="pw", tag="p1")
                for c in range(2):
                    nc.tensor.matmul(pw, vwT[c][:, ql * 128:(ql + 1) * 128],
                                     wo_sb[c][:, do * QB:(do + 1) * QB],
                                     start=(c == 0), stop=(c == 1))
                osb = os_pool.tile([128, QB], F32, name="osb", tag="osb")
                nc.vector.tensor_copy(osb, pw)
                nc.sync.dma_start(
                    out_d.ap()[qi * QB + ql * 128: qi * QB + (ql + 1) * 128,
                               do * QB:(do + 1) * QB], osb)


_COMPILED = None


def get_compiled():
    global _COMPILED
    if _COMPILED is not None:
        return _COMPILED
    nc = bacc.Bacc("TRN2", target_bir_lowering=False, debug=False,
                   enable_asserts=False, num_devices=NCORES)
    x_d = nc.dram_tensor("x", [S, D], F32, kind="ExternalInput")
    wqk_d = nc.dram_tensor("wqk", [D, FQK], R32, kind="ExternalInput")
    wv_d = nc.dram_tensor("wv", [D, FV], R32, kind="ExternalInput")
    bqk_d = nc.dram_tensor("bqk", [FQK], F32, kind="ExternalInput")
    bv_d = nc.dram_tensor("bv", [FV], R32, kind="ExternalInput")
    wo_d = nc.dram_tensor("wo", [FV, D], R32, kind="ExternalInput")
    out_d = nc.dram_tensor("out", [S, D], F32, kind="ExternalOutput")
    with tile.TileContext(nc) as tc:
        with ExitStack() as ctx:
            _build_body(ctx, tc, x_d, wqk_d, wv_d, bqk_d, bv_d, wo_d, out_d)
    nc.compile()
    _COMPILED = nc
    return nc


def make_in_maps(x, Wqkv, bqkv, Wo):
    x = np.ascontiguousarray(np.asarray(x, dtype=np.float32))
    Wqkv = np.asarray(Wqkv, dtype=np.float32)
    bqkv = np.asarray(bqkv, dtype=np.float32)
    Wo = np.asarray(Wo, dtype=np.float32)
    in_maps = []
    for c in range(NCORES):
        b, hg = divmod(c, NHG)
        qs = slice(hg * FV, (hg + 1) * FV)
        ks = slice(D + hg * FV, D + (hg + 1) * FV)
        vs = slice(2 * D + hg * FV, 2 * D + (hg + 1) * FV)
        in_maps.append({
            "x": np.ascontiguousarray(x[b]),
            "wqk": np.ascontiguousarray(
                np.concatenate([Wqkv[:, qs], Wqkv[:, ks]], axis=1)),
            "wv": np.ascontiguousarray(Wqkv[:, vs]),
            "bqk": np.ascontiguousarray(
                np.concatenate([bqkv[qs], bqkv[ks]])),
            "bv": np.ascontiguousarray(bqkv[vs]),
            "wo": np.ascontiguousarray(Wo[hg * FV:(hg + 1) * FV, :]),
        })
    return in_maps


def run_sharded(x, Wqkv, bqkv, Wo, bo, **spmd_kwargs):
    nc = get_compiled()
    in_maps = make_in_maps(x, Wqkv, bqkv, Wo)
    res = run_bass_kernel_spmd(nc, in_maps, core_ids=list(range(NCORES)),
                               **spmd_kwargs)
    out = np.zeros((B, S, D), np.float32)
    for c in range(NCORES):
        out[c // NHG] += res.results[c]["out"]
    out += np.asarray(bo, dtype=np.float32)
    return out, res


def kernel(x, mask, Wqkv, bqkv, Wo, bo):
    out, _ = run_sharded(x, Wqkv, bqkv, Wo, bo)
    return out


# revision 14
# speedup vs baseline: 1.3331x; 1.0696x over previous
"""Causal multi-head attention (B=2, S=2048, D=1024, H=16) on 8 trn2 cores.

Sharding: batch (2-way) x head-group (4-way) = 8 cores. Each core computes
QKV projection for its batch restricted to its 4 heads, causal attention,
and a row-parallel slice of the output projection; the host sums the 4
partial outputs per batch (the all-reduce of the row-parallel Wo matmul).

Per-core kernel (Tile framework, fp32 data / fp32r matmuls):
  - x [2048,1024] is PE-transposed on chip to x^T tiles (contract dim D on
    partitions).
  - Q,K are produced in [feat, seq] layout (rhs = x^T), V in [seq, feat]
    layout (lhsT = x^T) with an extra ones-column per head so the PV matmul
    also produces the softmax denominator.
  - Scores are computed transposed, S_T[key, q] = K_blk.T @ Q (N=512 so
    fp32r runs at full rate), exp on ScalarE (scale=1/8 folded in), causal
    masking via precomputed 0/1 staircase masks on the diagonal blocks.
  - PV: out_T[65, q] = V_aug.T @ exp(S_T), accumulated over key blocks; row
    64 is the denominator. Normalization multiplies by a reciprocal row
    broadcast across partitions with a K=1 outer-product matmul.
  - Wo: out[q, :] = sum_c vw_T_c.T @ Wo_c.
"""

import numpy as np
from contextlib import ExitStack

import concourse.bass as bass
import concourse.mybir as mybir
import concourse.tile as tile
from concourse import bacc
from concourse.bass_utils import run_bass_kernel_spmd
from concourse.masks import make_identity

B, S, D, H, HD = 2, 2048, 1024, 16, 64
NCORES = 8
NHG = 4                  # head groups (cores per batch)
NH = H // NHG            # 4 local heads
FQK = NH * HD * 2        # 512 local q+k features
FV = NH * HD             # 256 local v features
QB = 512                 # query block (attention outer tile)
KB = 128                 # key block
NSC = S // QB            # 4 seq chunks
R32 = mybir.dt.float32r
B16 = mybir.dt.bfloat16
F16 = mybir.dt.float16
F32 = mybir.dt.float32
EXP = mybir.ActivationFunctionType.Exp


def _build_body(ctx, tc, x_d, wqk_d, wv_d, bqk_d, bv_d, wo_d, out_d):
    nc = tc.nc

    const = ctx.enter_context(tc.tile_pool(name="const", bufs=1))
    wq_pool = ctx.enter_context(tc.tile_pool(name="wqp", bufs=8))
    wvp = ctx.enter_context(tc.tile_pool(name="wvp", bufs=8))
    wop = ctx.enter_context(tc.tile_pool(name="wop", bufs=2))
    xs_pool = ctx.enter_context(tc.tile_pool(name="xsp", bufs=6))
    xt_pool = ctx.enter_context(tc.tile_pool(name="xtp", bufs=10))
    qk_pool = ctx.enter_context(tc.tile_pool(name="qkp", bufs=1))
    v_pool = ctx.enter_context(tc.tile_pool(name="vp", bufs=16))
    exp_pool = ctx.enter_context(tc.tile_pool(name="ep", bufs=4))
    vw_pool = ctx.enter_context(tc.tile_pool(name="vwp", bufs=2))
    rc_pool = ctx.enter_context(tc.tile_pool(name="rcp", bufs=2))
    os_pool = ctx.enter_context(tc.tile_pool(name="osp", bufs=3))
    p1 = ctx.enter_context(tc.tile_pool(name="p1", bufs=2, space="PSUM"))
    ps = ctx.enter_context(tc.tile_pool(name="ps", bufs=2, space="PSUM"))
    po = ctx.enter_context(tc.tile_pool(name="po", bufs=2, space="PSUM"))

    # ---- constants ----
    ident = const.tile([128, 128], F16)
    make_identity(nc, ident)
    ones_row = const.tile([1, 128], R32)
    # Staircase causal masks for the 4 diagonal key-blocks of a 512-wide
    # query chunk: mask_j[k, q] = 1 iff q >= k + 128*j.
    masks = []
    for j in range(4):
        mj = const.tile([128, QB], F16, name=f"mask{j}", tag=f"mask{j}")
        nc.gpsimd.memset(mj, 1.0)
        nc.gpsimd.affine_select(
            out=mj, in_=mj,
            compare_op=mybir.AluOpType.is_ge,
            fill=0.0,
            base=-128 * j,
            pattern=[[1, QB]],
            channel_multiplier=-1,
        )
        masks.append(mj)
    # ones_row = ident*0 + 1 (memset can't write fp32r; tensor_scalar can)
    nc.vector.tensor_scalar(ones_row, ident[0:1, :], 0.0, 1.0,
                            op0=mybir.AluOpType.mult, op1=mybir.AluOpType.add)

    # ---- first x chunk before weights: PE transposes depend only on x ----
    xts0 = []
    for sb in range(4):
        xsb = xs_pool.tile([128, D], F16, name="xsb", tag="xsb")
        nc.sync.dma_start(xsb, x_d.ap()[sb * 128:(sb + 1) * 128, :])
        xts0.append(xsb)

    # ---- weights ----
    wqk_sb = []
    for dc in range(8):
        t = wq_pool.tile([128, FQK], F16, name=f"wqk{dc}", tag="wqk")
        nc.sync.dma_start(t, wqk_d.ap()[dc * 128:(dc + 1) * 128, :])
        wqk_sb.append(t)
    wv_sb = []
    for dc in range(8):
        t = wvp.tile([128, FV], F16, name=f"wv{dc}", tag="wv")
        nc.sync.dma_start(t, wv_d.ap()[dc * 128:(dc + 1) * 128, :])
        wv_sb.append(t)
    wo_sb = []
    for c in range(2):
        t = wop.tile([128, D], F16, name=f"wo{c}", tag="wo")
        nc.sync.dma_start(t, wo_d.ap()[c * 128:(c + 1) * 128, :])
        wo_sb.append(t)
    bqk_sb = const.tile([128, 4], F32)
    nc.sync.dma_start(bqk_sb, bqk_d.ap().rearrange("(f p) -> p f", p=128))
    bv_sb = const.tile([1, FV], R32)
    nc.sync.dma_start(bv_sb, bv_d.ap().rearrange("(o e) -> o e", o=1))
    # v-bias broadcast across partitions: [128, FV] = ones[1,128].T @ bv[1,FV]
    bvb_ps = p1.tile([128, FV], F32, name="bvb_ps", tag="p1")
    nc.tensor.matmul(bvb_ps, ones_row, bv_sb, start=True, stop=True)
    bvb_sb = const.tile([128, FV], F32)
    nc.vector.tensor_copy(bvb_sb, bvb_ps)

    # ---- phase B: x transpose + QKV projection ----
    qkT = [qk_pool.tile([128, S], F16, name=f"qkT{f}", tag=f"qkT{f}", bufs=1)
           for f in range(4)]
    v_tiles = []
    for sc in range(NSC):
        if sc == 0:
            xts = xts0
        else:
            xts = []
            for sb in range(4):
                xsb = xs_pool.tile([128, D], F16, name="xsb", tag="xsb")
                nc.sync.dma_start(
                    xsb,
                    x_d.ap()[sc * QB + sb * 128: sc * QB + (sb + 1) * 128, :])
                xts.append(xsb)
        xT = []
        for dc in range(8):
            xt = xt_pool.tile([128, QB], F16, name="xt", tag="xt")
            for sb in range(4):
                pt = p1.tile([128, 128], F16, name="pt", tag="p1")
                nc.tensor.transpose(pt, xts[sb][:, dc * 128:(dc + 1) * 128], ident)
                nc.vector.tensor_copy(xt[:, sb * 128:(sb + 1) * 128], pt)
            xT.append(xt)
        # Q,K in [feat, seq]: psum += Wqk_chunk.T @ x^T
        for f in range(4):
            pq = p1.tile([128, QB], F32, name="pq", tag="p1")
            for dc in range(8):
                nc.tensor.matmul(pq, wqk_sb[dc][:, f * 128:(f + 1) * 128],
                                 xT[dc], start=(dc == 0), stop=(dc == 7))
            nc.vector.tensor_scalar_add(
                qkT[f][:, sc * QB:(sc + 1) * QB], pq, bqk_sb[:, f:f + 1])
        # V in [seq, feat]: psum += (x^T_blk).T @ Wv_chunk, plus ones column
        for sb in range(4):
            pv = p1.tile([128, FV], F32, name="pv", tag="p1")
            for dc in range(8):
                nc.tensor.matmul(pv, xT[dc][:, sb * 128:(sb + 1) * 128],
                                 wv_sb[dc], start=(dc == 0), stop=(dc == 7))
            vt = v_pool.tile([128, NH, HD + 1], F16, name="vt", tag="vt")
            nc.vector.tensor_add(vt[:, :, 0:HD],
                                 pv.rearrange("p (h e) -> p h e", h=NH),
                                 bvb_sb.rearrange("p (h e) -> p h e", h=NH))
            nc.vector.tensor_scalar(vt[:, :, HD:HD + 1], vt[:, :, 0:1], 0.0,
                                    1.0, op0=mybir.AluOpType.mult,
                                    op1=mybir.AluOpType.add)
            v_tiles.append(vt)

    # ---- phase C: attention + output projection ----
    for qi in range(NSC):
        vwT = [vw_pool.tile([128, QB], F16, name=f"vwT{c}", tag=f"vwT{c}")
               for c in range(2)]
        for hp in range(2):
            pair = (2 * hp, 2 * hp + 1)
            nkb = (qi + 1) * 4
            poh, Q, Kt = {}, {}, {}
            for h in pair:
                poh[h] = po.tile([HD + 1, QB], F32, name="poh", tag="po")
                r0 = (h % 2) * 64
                Q[h] = qkT[h // 2][r0:r0 + 64, qi * QB:(qi + 1) * QB]
                Kt[h] = qkT[2 + h // 2][r0:r0 + 64, :]
            for base in range(0, nkb, 2):
                es = {}
                for h in pair:
                    psn = ps.tile([128, 2 * QB], F32, name="psn", tag="ps")
                    for j2 in range(2):
                        kb = base + j2
                        nc.tensor.matmul(psn[:, j2 * QB:(j2 + 1) * QB],
                                         Kt[h][:, kb * KB:(kb + 1) * KB],
                                         Q[h], start=True, stop=True)
                    e = exp_pool.tile([128, 2 * QB], F16, name="et", tag="et")
                    nc.scalar.activation(e, psn, EXP, scale=1.0 / np.sqrt(HD))
                    for j2 in range(2):
                        kb = base + j2
                        if kb >= qi * 4:
                            j = kb - qi * 4
                            nc.vector.tensor_mul(e[:, j2 * QB:(j2 + 1) * QB],
                                                 e[:, j2 * QB:(j2 + 1) * QB],
                                                 masks[j])
                    es[h] = e
                for j2 in range(2):
                    kb = base + j2
                    for h in pair:
                        nc.tensor.matmul(poh[h], v_tiles[kb][:, h, :],
                                         es[h][:, j2 * QB:(j2 + 1) * QB],
                                         start=(kb == 0), stop=(kb == nkb - 1))
            for h in pair:
                sum_sb = rc_pool.tile([1, QB], F32, name="sum_sb", tag="sum_sb")
                nc.vector.tensor_copy(sum_sb, poh[h][HD:HD + 1, :])
                rc = rc_pool.tile([1, QB], F32, name="rc", tag="rc")
                nc.vector.reciprocal_approx_fast(rc, sum_sb)
                rc32 = rc_pool.tile([1, QB], R32, name="rc32", tag="rc32")
                nc.vector.tensor_copy(rc32, rc)
                pb = p1.tile([64, QB], F32, name="pb", tag="p1")
                nc.tensor.matmul(pb, ones_row[:, 0:64], rc32,
                                 start=True, stop=True)
                bcs = rc_pool.tile([64, QB], F32, name="bcs", tag="bcs")
                nc.vector.tensor_copy(bcs, pb)
                r0 = (h % 2) * 64
                nc.vector.tensor_mul(vwT[h // 2][r0:r0 + 64, :],
                                     poh[h][0:HD, :], bcs)
        for ql in range(4):
            for do in range(2):
                pw = p1.tile([128, QB], F32, name="pw", tag="p1")
                for c in range(2):
                    nc.tensor.matmul(pw, vwT[c][:, ql * 128:(ql + 1) * 128],
                                     wo_sb[c][:, do * QB:(do + 1) * QB],
                                     start=(c == 0), stop=(c == 1))
                osb = os_pool.tile([128, QB], F32, name="osb", tag="osb")
                nc.vector.tensor_copy(osb, pw)
                nc.sync.dma_start(
                    out_d.ap()[qi * QB + ql * 128: qi * QB + (ql + 1) * 128,
                               do * QB:(do + 1) * QB], osb)


_COMPILED = None


def get_compiled():
    global _COMPILED
    if _COMPILED is not None:
        return _COMPILED
    nc = bacc.Bacc("TRN2", target_bir_lowering=False, debug=False,
                   enable_asserts=False, num_devices=NCORES)
    x_d = nc.dram_tensor("x", [S, D], F16, kind="ExternalInput")
    wqk_d = nc.dram_tensor("wqk", [D, FQK], F16, kind="ExternalInput")
    wv_d = nc.dram_tensor("wv", [D, FV], F16, kind="ExternalInput")
    bqk_d = nc.dram_tensor("bqk", [FQK], F32, kind="ExternalInput")
    bv_d = nc.dram_tensor("bv", [FV], R32, kind="ExternalInput")
    wo_d = nc.dram_tensor("wo", [FV, D], F16, kind="ExternalInput")
    out_d = nc.dram_tensor("out", [S, D], F32, kind="ExternalOutput")
    with tile.TileContext(nc) as tc:
        with ExitStack() as ctx:
            _build_body(ctx, tc, x_d, wqk_d, wv_d, bqk_d, bv_d, wo_d, out_d)
    nc.compile()
    _COMPILED = nc
    return nc


def make_in_maps(x, Wqkv, bqkv, Wo):
    x = np.ascontiguousarray(np.asarray(x, dtype=np.float32))
    Wqkv = np.asarray(Wqkv, dtype=np.float32)
    bqkv = np.asarray(bqkv, dtype=np.float32)
    Wo = np.asarray(Wo, dtype=np.float32)
    in_maps = []
    for c in range(NCORES):
        b, hg = divmod(c, NHG)
        qs = slice(hg * FV, (hg + 1) * FV)
        ks = slice(D + hg * FV, D + (hg + 1) * FV)
        vs = slice(2 * D + hg * FV, 2 * D + (hg + 1) * FV)
        in_maps.append({
            "x": np.ascontiguousarray(x[b]).astype(np.float16),
            "wqk": np.ascontiguousarray(
                np.concatenate([Wqkv[:, qs], Wqkv[:, ks]], axis=1)).astype(np.float16),
            "wv": np.ascontiguousarray(Wqkv[:, vs]).astype(np.float16),
            "bqk": np.ascontiguousarray(
                np.concatenate([bqkv[qs], bqkv[ks]])),
            "bv": np.ascontiguousarray(bqkv[vs]),
            "wo": np.ascontiguousarray(Wo[hg * FV:(hg + 1) * FV, :]).astype(np.float16),
        })
    return in_maps


def run_sharded(x, Wqkv, bqkv, Wo, bo, **spmd_kwargs):
    nc = get_compiled()
    in_maps = make_in_maps(x, Wqkv, bqkv, Wo)
    res = run_bass_kernel_spmd(nc, in_maps, core_ids=list(range(NCORES)),
                               **spmd_kwargs)
    out = np.zeros((B, S, D), np.float32)
    for c in range(NCORES):
        out[c // NHG] += res.results[c]["out"]
    out += np.asarray(bo, dtype=np.float32)
    return out, res


def kernel(x, mask, Wqkv, bqkv, Wo, bo):
    out, _ = run_sharded(x, Wqkv, bqkv, Wo, bo)
    return out
